# revision 10
# baseline (speedup 1.0000x reference)
"""Trainium2 Bass kernel for nn_L4maAttention (llama3.1-style GQA attention layer).

Sharding: heads across 8 cores (4 Q heads + 1 KV head per core), with
on-device collectives to minimize host<->device traffic:
  - hidden_states shipped hid-sharded ([512, 2048] bf16 per core) and
    AllGathered on device in 4 pipelined chunks; projections consume
    AG blocks as they land (PSUM partials spilled to SBUF f32 between
    contraction groups).
  - q/k/v projections column-parallel + rope on device.
  - paged-KV context gathered on host, shipped pre-transposed per core
    (1 kv head per core).
  - attention per-head local in S^T layout ([kv, q]); softmax
    denominators via an f32 SBUF accumulation of the exp tiles plus a
    single all-ones matmul per head.
  - o_proj COLUMN-parallel: per-batch AllGather of the (bf16) attention
    outputs, then each core computes a disjoint 512-column slice of the
    output. No reduction; outputs return as bf16 [512, 2048] per core.

All matmul operands are bf16 (1 cycle/row on the PE).
"""

import math
import sys

import numpy as np

sys.path.insert(0, "/opt/trn_rl_repo")

import concourse.bass as bass  # noqa: E402
import concourse.mybir as mybir  # noqa: E402
import concourse.tile as tile  # noqa: E402
from concourse import bacc  # noqa: E402
from concourse.bass_utils import run_bass_kernel_spmd  # noqa: E402
from concourse.masks import make_identity  # noqa: E402

# ---- problem constants (hardcoded from spec) ----
B, QO, PAGE = 4, 512, 16
HID, HQ, HKV, D = 4096, 32, 8, 128
N = B * QO  # 2048
NCORES = 8
HQL = HQ // NCORES  # 4 local q heads
ROPE_THETA = 5e5
OLD_CTX, LOW_F, HIGH_F, RSCALE = 8192.0, 1.0, 4.0, 8.0
SM_SCALE = 1.0 / math.sqrt(D)

import ml_dtypes  # noqa: E402

BF16NP = ml_dtypes.bfloat16
F32 = mybir.dt.float32
BF16 = mybir.dt.bfloat16
AF = mybir.ActivationFunctionType
ALU = mybir.AluOpType
P = 128
HS = HID // NCORES  # 512 hid rows per core for the h all-gather
JG = 4              # h all-gather split count
CW = 512            # projection moving-chunk width (tokens)
NCH = N // CW       # 4 chunks


def _llama31_inv_freq(d):
    inv = ROPE_THETA ** (-np.arange(0, d, 2, dtype=np.float32) / d)
    wavelen = 2.0 * np.pi / inv
    low_wl, high_wl = OLD_CTX / LOW_F, OLD_CTX / HIGH_F
    smooth = (OLD_CTX / wavelen - LOW_F) / (HIGH_F - LOW_F)
    mid = (1.0 - smooth) * inv / RSCALE + smooth * inv
    return np.where(
        wavelen > low_wl, inv / RSCALE, np.where(wavelen < high_wl, inv, mid)
    ).astype(np.float32)


def host_prep(inputs):
    """Shard + pre-transpose inputs for the 8 cores. Returns (in_maps, ctxl)."""
    hs = np.asarray(inputs["hidden_states"], np.float32)
    pos_ids = np.asarray(inputs["position_ids"], np.int32)
    kvc = np.asarray(inputs["kv_cache"], np.float32)
    kpi = np.asarray(inputs["kv_page_indices"], np.int32)
    kpp = np.asarray(inputs["kv_page_indptr"], np.int32)
    klp = np.asarray(inputs["kv_last_page_lens"], np.int32)
    qop = np.asarray(inputs["qo_indptr"], np.int32)
    Wq = np.asarray(inputs["Wq"], np.float32)
    Wk = np.asarray(inputs["Wk"], np.float32)
    Wv = np.asarray(inputs["Wv"], np.float32)
    Wo = np.asarray(inputs["Wo"], np.float32)

    n, hid = hs.shape
    b_sz = qop.shape[0] - 1
    qo_len = n // b_sz
    page = kvc.shape[2]
    pps = kpi.shape[0] // b_sz
    seq_len = (pps - 1) * page + klp  # [B]
    ctx_len = seq_len - qo_len
    assert n == N and hid == HID and b_sz == B and qo_len == QO
    assert np.all(ctx_len == ctx_len[0]) and int(ctx_len[0]) % 128 == 0
    ctxl = int(ctx_len[0])

    # rope tables [64, N] indexed (freq, token)
    inv = _llama31_inv_freq(D)
    ang = pos_ids.astype(np.float32)[:, None] * inv[None, :]
    cosT = np.ascontiguousarray(np.cos(ang).T).astype(np.float32)
    sinT = np.ascontiguousarray(np.sin(ang).T).astype(np.float32)

    # gather paged KV context (positions 0..ctxl-1 per sequence)
    cpos = np.arange(ctxl)
    pages = kpi[kpp[:-1][:, None] + (cpos[None, :] // page)]  # [B, ctxl]
    slots = np.broadcast_to(cpos % page, (b_sz, ctxl))
    Kc = kvc[pages, 0, slots]  # [B, ctxl, HKV, D]
    Vc = kvc[pages, 1, slots]

    # causal mask for the new-kv block, tiled [128, 4*512]: chunk c holds
    # rows kv_rel in [c*128,(c+1)*128) vs all 512 q_rel columns
    qr = np.arange(qo_len)
    mbig = np.where(qr[:, None] <= qr[None, :], 0.0, -1e30).astype(np.float32)
    msk = np.ascontiguousarray(
        np.concatenate([mbig[i * 128 : (i + 1) * 128] for i in range(qo_len // 128)], axis=1)
    )
    hT = np.ascontiguousarray(hs.T).astype(BF16NP)  # [HID, N]

    Wq4 = Wq.reshape(HQ, D, HID)
    Wk4 = Wk.reshape(HKV, D, HID)
    Wv4 = Wv.reshape(HKV, D, HID)

    in_maps = []
    for i in range(NCORES):
        hTs = np.ascontiguousarray(hT[i * HS : (i + 1) * HS, :])
        wqT = np.ascontiguousarray(Wq4[i * HQL : (i + 1) * HQL].reshape(HQL * D, HID).T).astype(BF16NP)
        wkT = np.ascontiguousarray(Wk4[i].T).astype(BF16NP)
        wvT = np.ascontiguousarray(Wv4[i].T).astype(BF16NP)
        # column-parallel o_proj slice: out cols [i*512,(i+1)*512), all head dims
        woTc = np.ascontiguousarray(Wo[i * QO : (i + 1) * QO, :].T).astype(BF16NP)  # [HQ*D, 512]
        kctxT = np.ascontiguousarray(Kc[:, :, i, :].reshape(b_sz * ctxl, D).T).astype(BF16NP)
        vctx = np.ascontiguousarray(
            Vc[:, :, i, :].reshape(-1, 128, D).transpose(1, 0, 2).reshape(128, b_sz * ctxl)
        ).astype(BF16NP)
        in_maps.append(
            dict(hTs=hTs, wqT=wqT, wkT=wkT, wvT=wvT, woTc=woTc, kctxT=kctxT,
                 vctx=vctx, cosT=cosT, sinT=sinT, msk=msk)
        )
    return in_maps, ctxl


def _rope_evict(nc, tpool, dst, src, cs, sn, w):
    """dst[0:64] = p1*cos - p2*sin ; dst[64:128] = p2*cos + p1*sin.

    src is SBUF, so each input pair must share a start partition: cs/sn
    hold the rope table duplicated in both partition halves.
    """
    t1 = tpool.tile([64, w], F32, tag="t1")
    t2 = tpool.tile([64, w], F32, tag="t2")
    t3 = tpool.tile([64, w], F32, tag="t3")
    t4 = tpool.tile([64, w], F32, tag="t4")
    nc.vector.tensor_tensor(t1[:], src[0:64, :], cs[0:64, :], ALU.mult)
    nc.vector.tensor_tensor(t2[:], src[64:128, :], sn[64:128, :], ALU.mult)
    nc.vector.tensor_tensor(dst[0:64, :], t1[:], t2[:], ALU.subtract)
    nc.vector.tensor_tensor(t3[:], src[64:128, :], cs[64:128, :], ALU.mult)
    nc.vector.tensor_tensor(t4[:], src[0:64, :], sn[0:64, :], ALU.mult)
    nc.vector.tensor_tensor(dst[64:128, :], t3[:], t4[:], ALU.add)


def build_program(ctxl):
    KVL = ctxl + QO  # kv length per sequence
    CC = ctxl // 128  # context chunks per sequence
    KC = KVL // 128  # total kv chunks per sequence
    KH = HID // 128  # contraction chunks for projections (32)

    nc = bacc.Bacc("TRN2", debug=False, num_devices=NCORES)
    hTs = nc.dram_tensor("hTs", [HS, N], BF16, kind="ExternalInput").ap()
    wqT = nc.dram_tensor("wqT", [HID, HQL * D], BF16, kind="ExternalInput").ap()
    wkT = nc.dram_tensor("wkT", [HID, D], BF16, kind="ExternalInput").ap()
    wvT = nc.dram_tensor("wvT", [HID, D], BF16, kind="ExternalInput").ap()
    woTc = nc.dram_tensor("woTc", [HQ * D, QO], BF16, kind="ExternalInput").ap()
    kctxT = nc.dram_tensor("kctxT", [D, B * ctxl], BF16, kind="ExternalInput").ap()
    vctx = nc.dram_tensor("vctx", [P, B * ctxl], BF16, kind="ExternalInput").ap()
    cosT = nc.dram_tensor("cosT", [D // 2, N], F32, kind="ExternalInput").ap()
    sinT = nc.dram_tensor("sinT", [D // 2, N], F32, kind="ExternalInput").ap()
    msk = nc.dram_tensor("msk", [P, (QO // 128) * QO], F32, kind="ExternalInput").ap()
    outT = nc.dram_tensor("outT", [QO, N], BF16, kind="ExternalOutput").ap()
    ones_c = nc.inline_tensor(np.ones((P, P), BF16NP), name="ones_c").ap()

    rg = [list(range(NCORES))]

    with tile.TileContext(nc) as tc:
        with tc.tile_pool(name="dram", bufs=1, space="DRAM") as dram:
            # ---- h all-gather, split into JG chunks along hid ----
            hins = [dram.tile([HS // JG, N], BF16, tag=f"hin{j}", name=f"hin{j}") for j in range(JG)]
            houts = [dram.tile([NCORES * (HS // JG), N], BF16, addr_space="Shared",
                               tag=f"hout{j}", name=f"hout{j}") for j in range(JG)]
            for j in range(JG):
                nc.sync.dma_start(hins[j][:], hTs[j * (HS // JG):(j + 1) * (HS // JG), :])
                nc.gpsimd.collective_compute(
                    "AllGather", ALU.bypass, replica_groups=rg,
                    ins=[hins[j][:]], outs=[houts[j][:]])
            # ---- per-batch O all-gather buffers ----
            oins = [dram.tile([HQL * D, QO], BF16, tag=f"oin{b}", name=f"oin{b}") for b in range(B)]
            oouts = [dram.tile([HQ * D, QO], BF16, addr_space="Shared",
                               tag=f"oout{b}", name=f"oout{b}") for b in range(B)]

            with tc.tile_pool(name="resident", bufs=1) as res:
                q_sb = res.tile([P, HQL * N], BF16)  # head h at cols [h*N,(h+1)*N)
                kn_sb = res.tile([P, N], BF16)  # new K^T, chunk c at cols c*CW
                vn_sb = res.tile([P, N], BF16)  # new V, 128-block t at cols t*128
                cos_sb = res.tile([P, N], F32)
                sin_sb = res.tile([P, N], F32)
                ones_sb = res.tile([P, P], BF16)
                ident = res.tile([P, P], BF16)
                nc.sync.dma_start(cos_sb[0:64, :], cosT)
                nc.sync.dma_start(cos_sb[64:128, :], cosT)
                nc.sync.dma_start(sin_sb[0:64, :], sinT)
                nc.sync.dma_start(sin_sb[64:128, :], sinT)
                nc.sync.dma_start(ones_sb[:], ones_c)
                make_identity(nc, ident[:])

                # ============ Phase A: QKV projections + rope ============
                with tc.tile_pool(name="wsb", bufs=1) as wpool, \
                     tc.tile_pool(name="accsb", bufs=1) as accpool, \
                     tc.tile_pool(name="hstream", bufs=6) as hpool, \
                     tc.tile_pool(name="qkvpsum", bufs=1, space="PSUM") as ppool, \
                     tc.tile_pool(name="vtpsum", bufs=2, space="PSUM") as vtpool, \
                     tc.tile_pool(name="ropetmp", bufs=2) as tpool, \
                     tc.tile_pool(name="vsb", bufs=2) as vsbpool:
                    wq_sb = wpool.tile([P, KH * HQL * D], BF16)  # (kt,m) at kt*512+m*128
                    wk_sb = wpool.tile([P, KH * D], BF16)
                    wv_sb = wpool.tile([P, KH * D], BF16)
                    for kt in range(KH):
                        nc.sync.dma_start(wq_sb[:, kt * 512:(kt + 1) * 512],
                                          wqT[kt * 128:(kt + 1) * 128, :])
                        nc.sync.dma_start(wk_sb[:, kt * 128:(kt + 1) * 128],
                                          wkT[kt * 128:(kt + 1) * 128, :])
                        nc.sync.dma_start(wv_sb[:, kt * 128:(kt + 1) * 128],
                                          wvT[kt * 128:(kt + 1) * 128, :])
                    # f32 accumulators: (chunk c, m) at cols (c*6+m)*CW
                    acc = accpool.tile([P, NCH * 6 * CW], F32)
                    RPB = HS // JG // 128  # rows (128-tiles) each rank contributes per AG chunk
                    for j in range(JG):
                        for c in range(NCH):
                            ps = [ppool.tile([P, CW], F32, tag=f"m{m}", name=f"ps{m}") for m in range(6)]
                            for r in range(NCORES):
                                for jj in range(RPB):
                                    kt = r * (HS // 128) + j * RPB + jj
                                    mov = hpool.tile([P, CW], BF16)
                                    nc.sync.dma_start(
                                        mov[:],
                                        houts[j][(r * RPB + jj) * 128:(r * RPB + jj + 1) * 128,
                                                 c * CW:(c + 1) * CW])
                                    st, sp = (r == 0 and jj == 0), (r == NCORES - 1 and jj == RPB - 1)
                                    for m in range(HQL):
                                        nc.tensor.matmul(
                                            ps[m][:],
                                            wq_sb[:, kt * 512 + m * 128: kt * 512 + (m + 1) * 128],
                                            mov[:], start=st, stop=sp)
                                    nc.tensor.matmul(
                                        ps[4][:], wk_sb[:, kt * 128:(kt + 1) * 128],
                                        mov[:], start=st, stop=sp)
                                    nc.tensor.matmul(
                                        ps[5][:], wv_sb[:, kt * 128:(kt + 1) * 128],
                                        mov[:], start=st, stop=sp)
                            for m in range(6):
                                a = acc[:, (c * 6 + m) * CW:(c * 6 + m + 1) * CW]
                                if j == 0:
                                    nc.scalar.activation(a, ps[m][:], AF.Copy)
                                else:
                                    nc.vector.tensor_tensor(a, a, ps[m][:], ALU.add)
                            if j == JG - 1:
                                cs = cos_sb[:, c * CW:(c + 1) * CW]
                                sn = sin_sb[:, c * CW:(c + 1) * CW]
                                for m in range(HQL):
                                    _rope_evict(
                                        nc, tpool,
                                        q_sb[:, m * N + c * CW: m * N + (c + 1) * CW],
                                        acc[:, (c * 6 + m) * CW:(c * 6 + m + 1) * CW],
                                        cs, sn, CW)
                                _rope_evict(
                                    nc, tpool, kn_sb[:, c * CW:(c + 1) * CW],
                                    acc[:, (c * 6 + 4) * CW:(c * 6 + 4 + 1) * CW],
                                    cs, sn, CW)
                                vt = vsbpool.tile([P, CW], BF16)
                                nc.scalar.activation(
                                    vt[:], acc[:, (c * 6 + 5) * CW:(c * 6 + 5 + 1) * CW],
                                    AF.Copy)
                                for t in range(CW // 128):
                                    tp = vtpool.tile([P, P], BF16)
                                    nc.tensor.transpose(
                                        tp[:], vt[:, t * 128:(t + 1) * 128], ident[:])
                                    nc.scalar.activation(
                                        vn_sb[:, (c * (CW // 128) + t) * 128:
                                              (c * (CW // 128) + t + 1) * 128],
                                        tp[:], AF.Copy)

                # ============ Phase B: attention (+ per-batch O AG) ============
                # ============ Phase C: column-parallel o_proj ============
                with tc.tile_pool(name="kvsb", bufs=1) as kvpool, \
                     tc.tile_pool(name="spsum", bufs=2, space="PSUM") as spool, \
                     tc.tile_pool(name="opsum", bufs=2, space="PSUM") as opool, \
                     tc.tile_pool(name="dpsum", bufs=2, space="PSUM") as dpool, \
                     tc.tile_pool(name="cpsum", bufs=2, space="PSUM") as cpool, \
                     tc.tile_pool(name="ptile", bufs=3) as p2pool, \
                     tc.tile_pool(name="accexp", bufs=2) as aepool, \
                     tc.tile_pool(name="rtile", bufs=2) as rpool, \
                     tc.tile_pool(name="osb", bufs=4) as osbpool, \
                     tc.tile_pool(name="wosb", bufs=1) as wopool, \
                     tc.tile_pool(name="ovstream", bufs=1) as ovpool, \
                     tc.tile_pool(name="outsb", bufs=3) as outpool:
                    kctx_sb = kvpool.tile([P, B * ctxl], BF16)
                    vctx_sb = kvpool.tile([P, B * ctxl], BF16)
                    msk_sb = kvpool.tile([P, (QO // 128) * QO], F32)
                    nc.sync.dma_start(kctx_sb[:], kctxT)
                    nc.sync.dma_start(vctx_sb[:], vctx)
                    nc.sync.dma_start(msk_sb[:], msk)
                    woc_sb = wopool.tile([P, KH * QO], BF16)  # (kt,ob) at kt*512+ob*128
                    for kt in range(KH):
                        nc.sync.dma_start(woc_sb[:, kt * 512:(kt + 1) * 512],
                                          woTc[kt * 128:(kt + 1) * 128, :])

                    def attn_batch(b):
                        for h in range(HQL):
                            po = opool.tile([P, QO], F32)
                            ae = aepool.tile([P, QO], F32)
                            qap = q_sb[:, h * N + b * QO: h * N + (b + 1) * QO]
                            for ckv in range(KC):
                                if ckv < CC:
                                    kl = kctx_sb[:, b * ctxl + ckv * 128: b * ctxl + (ckv + 1) * 128]
                                    vl = vctx_sb[:, b * ctxl + ckv * 128: b * ctxl + (ckv + 1) * 128]
                                else:
                                    jn = ckv - CC
                                    kl = kn_sb[:, b * QO + jn * 128: b * QO + (jn + 1) * 128]
                                    vl = vn_sb[:, (b * 4 + jn) * 128: (b * 4 + jn + 1) * 128]
                                st = spool.tile([P, QO], F32)
                                nc.tensor.matmul(st[:], kl, qap, start=True, stop=True)
                                if ckv >= CC:
                                    jn = ckv - CC
                                    nc.vector.tensor_tensor(
                                        st[:], st[:], msk_sb[:, jn * QO:(jn + 1) * QO],
                                        ALU.add)
                                pt = p2pool.tile([P, QO], BF16)
                                nc.scalar.activation(pt[:], st[:], AF.Exp, scale=SM_SCALE)
                                nc.tensor.matmul(po[:], vl, pt[:],
                                                 start=(ckv == 0), stop=(ckv == KC - 1))
                                if ckv == 0:
                                    nc.scalar.activation(ae[:], pt[:], AF.Copy)
                                else:
                                    nc.vector.tensor_tensor(ae[:], ae[:], pt[:], ALU.add)
                            aeb = rpool.tile([P, QO], BF16, tag="aeb")
                            nc.scalar.activation(aeb[:], ae[:], AF.Copy)
                            pd = dpool.tile([P, QO], F32)
                            nc.tensor.matmul(pd[:], ones_sb[:], aeb[:], start=True, stop=True)
                            rsb = rpool.tile([P, QO], F32, tag="rsb")
                            nc.vector.reciprocal(rsb[:], pd[:])
                            ot = osbpool.tile([P, QO], BF16)
                            nc.vector.tensor_tensor(ot[:], po[:], rsb[:], ALU.mult)
                            nc.sync.dma_start(
                                oins[b][h * 128:(h + 1) * 128, :], ot[:])
                        nc.gpsimd.collective_compute(
                            "AllGather", ALU.bypass, replica_groups=rg,
                            ins=[oins[b][:]], outs=[oouts[b][:]])

                    def oproj_batch(b):
                        ovs = []
                        for kt in range(KH):
                            ov = ovpool.tile([P, QO], BF16, name=f"ov{kt}")
                            nc.sync.dma_start(ov[:], oouts[b][kt * 128:(kt + 1) * 128, :])
                            ovs.append(ov)
                        for ob in range(QO // 128):
                            pc = cpool.tile([P, QO], F32)
                            for kt in range(KH):
                                nc.tensor.matmul(
                                    pc[:], woc_sb[:, kt * 512 + ob * 128: kt * 512 + (ob + 1) * 128],
                                    ovs[kt][:], start=(kt == 0), stop=(kt == KH - 1))
                            ot2 = outpool.tile([P, QO], BF16)
                            nc.scalar.activation(ot2[:], pc[:], AF.Copy)
                            nc.sync.dma_start(
                                outT[ob * 128:(ob + 1) * 128, b * QO:(b + 1) * QO], ot2[:])

                    for b in range(B):
                        attn_batch(b)
                        if b >= 1:
                            oproj_batch(b - 1)
                    oproj_batch(B - 1)
    nc.compile()
    return nc


_NC_CACHE = {}


def _get_program(ctxl):
    if ctxl not in _NC_CACHE:
        _NC_CACHE[ctxl] = build_program(ctxl)
    return _NC_CACHE[ctxl]


def run(inputs, trace=False):
    in_maps, ctxl = host_prep(inputs)
    nc = _get_program(ctxl)
    kw = dict(tmpdir="/tmp/trace_out") if trace else {}
    res = run_bass_kernel_spmd(nc, in_maps, core_ids=list(range(NCORES)), trace=trace, **kw)
    out = np.empty((N, HID), np.float32)
    for i, r in enumerate(res.results):
        out[:, i * QO:(i + 1) * QO] = np.asarray(r["outT"]).T.astype(np.float32)
    return out, res


def kernel(**inputs) -> np.ndarray:
    out, _ = run(inputs, trace=False)
    return out
